# revision 42
# baseline (speedup 1.0000x reference)
"""AdaptiveEmbedding forward on 8 TRN2 NeuronCores (Bass/Tile, SPMD).

Strategy (routing-as-sharding, 3 gathers per core):
  - Each SWDGE dma_gather costs ~1us of fixed Q7 descriptor-generation
    time, so the design minimizes gather ops: one head gather, one tail0
    gather, one tail1 gather per core.
  - Per-core table variants make that possible under the int16 index
    limit (<=32768 rows per gatherable view):
      * tail0 (60000 rows x 256 f16): core c holds half c%2
        ([30000, 256] f16, 512B rows).
      * tail1 (120000 rows x 64): packed 2-wide with a residue shift --
        variant (h, r) = rows [60000h + r : +60000] of the f16 table
        reshaped [30000, 128] (256B rows). A token t with
        (t-80000) % 2 == r and half h gathers super-row
        ((t-80000) - 60000h) // 2, and its 64-value embedding lands at
        SBUF partitions 0:64 on EVERY core (the shift absorbs the
        residue), keeping the program SPMD-uniform.
    Tokens are routed host-side to a core that holds their rows; head
    tokens (all cores hold the head table) balance per-core load.
  - Head cluster: projection folded into the table host-side
    (tproj = table0 @ proj0 * 32, f16); head tokens are a pure 2KB-row
    gather that goes straight back out to DRAM.
  - tail0: transposed f16 gather lands [256-K x token]; 2-k-tile f16
    matmuls against the pre-scaled projection; f32 PSUM.
  - tail1: transposed f16 gather lands K=64 x token directly (no PE
    transpose / repack needed); K=64 matmuls.
  - Output rows are packed back-to-back per segment (no 128-row
    padding); drains are emitted right after each group's PSUM copies
    so output DMA overlaps the remaining compute. The host
    inverse-permutes + upcasts to the [8, 1024, 1024] f32 result.
"""

import numpy as np

import concourse.bass as bass
import concourse.tile as tile
from concourse import bacc, mybir
from concourse.bass_utils import run_bass_kernel_spmd

HIDDEN = 1024
N_CORES = 8
HEAD_V = 20000
T0_LO, T1_LO = 20000, 80000
T0_ROWS, T1_ROWS = 60000, 120000

F32 = mybir.dt.float32
F16 = mybir.dt.float16
I16 = mybir.dt.int16

OUT_NP = np.float16

# A/B knobs (env-overridable for benching; defaults are the shipped config)
import os as _os
K_WARM_MM = int(_os.environ.get("K_WARM_MM", "14"))
K_UNROLL = int(_os.environ.get("K_UNROLL", "8"))
K_SINGLE_PACKET = bool(int(_os.environ.get("K_SINGLE_PACKET", "1")))
K_STAGGER = int(_os.environ.get("K_STAGGER", "0"))
K_SCRATCH = int(_os.environ.get("K_SCRATCH", "16384"))
K_COPY_ALT = int(_os.environ.get("K_COPY_ALT", "0"))
K_PROBE = _os.environ.get("K_PROBE", "")  # "", "gathers", "drains", "compute"
# filler matmuls per body copy: dependency-free PE work that bridges the
# gather-wait gaps so the PE clock gate never drops out of full speed
K_FILL = int(_os.environ.get("K_FILL", "6"))
# rotate gather queue assignment per body copy so one queue's SWDGE ring
# doesn't hold every t1 gather's descriptors (640 each, ring ~1024)
K_QROT = int(_os.environ.get("K_QROT", "0"))


def _ceil(a, m):
    return -(-a // m) * m


def _plan(x):
    """Host-side routing: per-(segment, core) token lists + idx packing.

    Core c serves: head (any), tail0 half c%2, tail1 variant c//2
    (= half (c//2)//2, residue (c//2)%2).
    """
    flat = np.asarray(x).reshape(-1).astype(np.int64)

    seg_tok = [[None] * N_CORES for _ in range(3)]  # head, t0, t1
    # tail0: half h -> cores h, h+2, h+4, h+6 (round-robin)
    for h in range(2):
        sel = np.nonzero((flat >= T0_LO) & (flat < T1_LO)
                         & (((flat - T0_LO) // 30000) == h))[0]
        sel = sel[np.argsort(flat[sel], kind="stable")]
        for i in range(4):
            seg_tok[1][h + 2 * i] = sel[i::4]
    # tail1: variant v=(h,r) -> cores 2v, 2v+1
    for v in range(4):
        h, r = v // 2, v % 2
        tl = flat - T1_LO
        sel = np.nonzero((flat >= T1_LO) & (tl // 60000 == h)
                         & (tl % 2 == r))[0]
        sel = sel[np.argsort(flat[sel], kind="stable")]
        for i in range(2):
            seg_tok[2][2 * v + i] = sel[i::2]
    # head: balance per-core totals (waterfill to the least-loaded cores)
    hsel = np.nonzero(flat < HEAD_V)[0]
    hsel = hsel[np.argsort(flat[hsel], kind="stable")]
    loads = np.array([len(seg_tok[1][c]) + len(seg_tok[2][c])
                      for c in range(N_CORES)], np.int64)
    counts = np.zeros(N_CORES, np.int64)
    lo, hi = 0, int(loads.max() + len(hsel))
    while lo < hi:  # max level L s.t. sum(max(0, L - loads)) <= n_head
        mid = (lo + hi + 1) // 2
        if np.maximum(0, mid - loads).sum() <= len(hsel):
            lo = mid
        else:
            hi = mid - 1
    counts = np.maximum(0, lo - loads)
    rem = len(hsel) - counts.sum()
    counts[np.argsort(loads + counts)[:rem]] += 1
    off = np.cumsum(np.concatenate([[0], counts]))
    for c in range(N_CORES):
        seg_tok[0][c] = hsel[off[c]:off[c + 1]]

    nmax = [max(len(seg_tok[s][c]) for c in range(N_CORES)) for s in range(3)]
    npad = [_ceil(max(n, 1), 128) for n in nmax]
    off16 = np.cumsum([0] + [p // 16 for p in npad])
    tot16 = int(off16[-1])

    idx_arrs = []
    for c in range(N_CORES):
        arr = np.zeros((16, tot16), np.int16)
        arr[:, :npad[0] // 16] = -1  # head pad: negative idx -> row skipped
        for s in range(3):
            toks = seg_tok[s][c]
            ids = flat[toks]
            if s == 0:
                ids = ids
            elif s == 1:
                ids = ids - T0_LO - 30000 * (c % 2)
            else:
                v = c // 2
                ids = (ids - T1_LO - 60000 * (v // 2)) // 2
            ids = ids.astype(np.int16)
            i = np.arange(len(ids))
            arr[i % 16, int(off16[s]) + i // 16] = ids
        # the GPSIMD Q7 cores each read their own 16-partition stripe:
        # the wrapped index pattern must be replicated across all 8 stripes
        idx_arrs.append(np.tile(arr, (8, 1)))

    row_base = [0, nmax[0], nmax[0] + nmax[1]]
    tot_rows = nmax[0] + nmax[1] + nmax[2]

    return dict(
        seg_tok=seg_tok, nmax=nmax, npad=npad,
        off16=[int(v) for v in off16], tot16=tot16,
        row_base=row_base, tot_rows=tot_rows, idx=idx_arrs,
    )


def _emit_const(nc, tc, ctx, P, T):
    const = ctx.enter_context(tc.tile_pool(name="const", bufs=1))
    idx_sb = const.tile([128, P["tot16"]], I16, tag="idx")
    nc.sync.dma_start(idx_sb[:], T["idx"].ap()[:])
    p1_sb = const.tile([128, 2, HIDDEN], F16, tag="p1")
    nc.scalar.dma_start(p1_sb[:], T["p1"].ap()[:])
    p2_sb = const.tile([64, HIDDEN], F16, tag="p2")
    nc.scalar.dma_start(p2_sb[:], T["p2"].ap()[:])
    warm = const.tile([64, 512], F16, tag="warm")
    nc.vector.memset(warm[:], 0.0)
    # PE pre-warm: the clock gate holds the array at reduced speed until it
    # sees sustained activity; burn the initial gather window on dummy
    # matmuls so the first real matmul burst starts near full clock.  Runs
    # once, before the repeat loop.
    psum_w = ctx.enter_context(tc.tile_pool(name="psum_w", bufs=1, space="PSUM"))
    for w in range(K_WARM_MM):
        wps = psum_w.tile([128, 512], F32, tag="warm", name=f"wps{w}")
        nc.tensor.matmul(wps[:, :], warm[:, 0:128], warm[:, :],
                         start=True, stop=True)
    return dict(idx=idx_sb, p1=p1_sb, p2=p2_sb, warm=warm, psum_w=psum_w)


def _emit_body(nc, tc, ctx, P, T, C, pools, sfx="", out_key="out0"):
    nmax, npad, off16 = P["nmax"], P["npad"], P["off16"]
    row_base = P["row_base"]
    bufs, outp, psum_mm = pools
    idx_sb, p1_sb, p2_sb = C["idx"], C["p1"], C["p2"]
    out = T[out_key].ap()

    # ---- phase 1: the three gathers, longest consumer chain first ----
    g1 = bufs.tile([128, 1, npad[2]], F16, tag=f"g1{sfx}", name=f"g1{sfx}")
    e1 = bufs.tile([128, 2, npad[1]], F16, tag=f"e1{sfx}", name=f"e1{sfx}")
    hb = bufs.tile([128, npad[0] // 128, HIDDEN], F16, tag=f"hb{sfx}",
                   name=f"hb{sfx}")
    if K_PROBE != "drains":
        ucopy = int(sfx[2:]) if sfx else 0
        qb = (3 * ucopy) % 4 if K_QROT else 0
        nc.gpsimd.dma_gather(
            g1[:], T["t1v"].ap()[:], idx_sb[:, off16[2]:off16[3]],
            num_idxs=npad[2], num_idxs_reg=npad[2], elem_size=128,
            transpose=True, queue_num=qb, single_packet=K_SINGLE_PACKET,
        )
        nc.gpsimd.dma_gather(
            e1[:], T["t0b"].ap()[:], idx_sb[:, off16[1]:off16[2]],
            num_idxs=npad[1], num_idxs_reg=npad[1], elem_size=256,
            transpose=True, queue_num=(qb + 1) % 4,
            single_packet=K_SINGLE_PACKET,
        )
        nc.gpsimd.dma_gather(
            hb[:], T["tproj"].ap()[:], idx_sb[:, off16[0]:off16[1]],
            num_idxs=npad[0], num_idxs_reg=npad[0], elem_size=HIDDEN,
            queue_num=(qb + 2) % 4, single_packet=K_SINGLE_PACKET,
        )
    if K_PROBE == "gathers":
        return

    for w in range(K_FILL):
        wps = C["psum_w"].tile([128, 512], F32, tag="warm",
                               name=f"fill{sfx}_{w}")
        nc.tensor.matmul(wps[:, :], C["warm"][:, 0:128], C["warm"][:, :],
                         start=True, stop=True)

    dma_eng = [nc.sync, nc.scalar]
    ndma = [0]

    def drain(seg_base, r0, st, g0, g1_, vrows):
        # rows [r0, r0+vrows) of out <- stage groups [g0, g1_)
        if K_PROBE == "compute":
            return
        if vrows == (g1_ - g0) * 128:
            dst = out[seg_base + r0:seg_base + r0 + vrows, :].rearrange(
                "(g p) h -> p g h", p=128)
            dma_eng[ndma[0] % 2].dma_start(dst, st[:, g0:g1_, :])
        else:
            dma_eng[ndma[0] % 2].dma_start(
                out[seg_base + r0:seg_base + r0 + vrows, :],
                st[0:vrows, g0, :])
        ndma[0] += 1

    # ---- head rows go straight out ----
    for g in range(npad[0] // 128):
        v = min(128, nmax[0] - 128 * g)
        if v <= 0 or K_PROBE == "compute":
            break
        dma_eng[ndma[0] % 2].dma_start(
            out[row_base[0] + 128 * g:row_base[0] + 128 * g + v, :],
            hb[0:v, g, :])
        ndma[0] += 1

    # ---- tail1 matmuls (K=64), drains emitted pairwise as groups land ----
    ng2 = npad[2] // 128
    st1 = outp.tile([128, ng2, HIDDEN], F16, tag=f"st1{sfx}", name=f"st1{sfx}")
    for m in range(ng2):
        if K_PROBE != "drains":
            psa = psum_mm.tile([128, 512], F32, tag="mm", name="psa")
            psb = psum_mm.tile([128, 512], F32, tag="mm", name="psb")
            for nt, ps in enumerate((psa, psb)):
                nc.tensor.matmul(
                    ps[:, :],
                    g1[0:64, 0, 128 * m:128 * (m + 1)],
                    p2_sb[:, nt * 512:(nt + 1) * 512],
                    start=True, stop=True,
                )
            nc.vector.tensor_copy(st1[:, m, 0:512], psa[:, :])
            nc.scalar.copy(st1[:, m, 512:1024], psb[:, :])
        if m % 2 == 1:
            vr = min(nmax[2], 128 * (m + 1)) - 128 * (m - 1)
            if vr > 0:
                if vr >= 256:
                    drain(row_base[2], 128 * (m - 1), st1, m - 1, m + 1, 256)
                else:
                    if vr > 128:
                        drain(row_base[2], 128 * (m - 1), st1, m - 1, m, 128)
                        vr -= 128
                        drain(row_base[2], 128 * m, st1, m, m + 1, vr)
                    else:
                        drain(row_base[2], 128 * (m - 1), st1, m - 1, m, vr)
    if ng2 % 2 == 1:
        vr = nmax[2] - 128 * (ng2 - 1)
        if vr > 0:
            drain(row_base[2], 128 * (ng2 - 1), st1, ng2 - 1, ng2, vr)

    # ---- tail0 matmuls (K=256 via 2 k-tiles) ----
    ng1 = npad[1] // 128
    st0 = outp.tile([128, ng1, HIDDEN], F16, tag=f"st0{sfx}", name=f"st0{sfx}")
    for g in range(ng1):
        if K_PROBE != "drains":
            psa = psum_mm.tile([128, 512], F32, tag="mm", name="psa")
            psb = psum_mm.tile([128, 512], F32, tag="mm", name="psb")
            for kt in range(2):
                for nt, ps in enumerate((psa, psb)):
                    nc.tensor.matmul(
                        ps[:, :],
                        e1[:, kt, 128 * g:128 * (g + 1)],
                        p1_sb[:, kt, nt * 512:(nt + 1) * 512],
                        start=(kt == 0), stop=(kt == 1),
                    )
            nc.vector.tensor_copy(st0[:, g, 0:512], psa[:, :])
            nc.scalar.copy(st0[:, g, 512:1024], psb[:, :])
        if g % 2 == 1:
            vr = min(nmax[1], 128 * (g + 1)) - 128 * (g - 1)
            if vr > 0:
                if vr >= 256:
                    drain(row_base[1], 128 * (g - 1), st0, g - 1, g + 1, 256)
                else:
                    if vr > 128:
                        drain(row_base[1], 128 * (g - 1), st0, g - 1, g, 128)
                        vr -= 128
                        drain(row_base[1], 128 * g, st0, g, g + 1, vr)
                    else:
                        drain(row_base[1], 128 * (g - 1), st0, g - 1, g, vr)
    if ng1 % 2 == 1:
        vr = nmax[1] - 128 * (ng1 - 1)
        if vr > 0:
            drain(row_base[1], 128 * (ng1 - 1), st0, ng1 - 1, ng1, vr)


def _build(P, repeat=1):
    import contextlib
    nc = bacc.Bacc("TRN2", target_bir_lowering=False, debug=False,
                   num_devices=N_CORES, num_swdge_queues=4,
                   dynamic_dma_scratch_size=K_SCRATCH)
    unroll = 1 if repeat == 1 else (K_UNROLL if repeat % K_UNROLL == 0 else 1)
    T = dict(
        tproj=nc.dram_tensor("tproj", [HEAD_V, HIDDEN], F16, kind="ExternalInput"),
        t0b=nc.dram_tensor("t0b", [30000, 256], F16, kind="ExternalInput"),
        t1v=nc.dram_tensor("t1v", [30000, 128], F16, kind="ExternalInput"),
        p1=nc.dram_tensor("p1", [128, 2, HIDDEN], F16, kind="ExternalInput"),
        p2=nc.dram_tensor("p2", [64, HIDDEN], F16, kind="ExternalInput"),
        idx=nc.dram_tensor("idx", [128, P["tot16"]], I16, kind="ExternalInput"),
    )
    # per-copy output buffers: every unrolled copy computes the same result
    # into its own buffer, so drains of successive copies/iterations carry no
    # write-after-write dependency and pipeline freely.  The host reads out0.
    for u in range(unroll):
        T[f"out{u}"] = nc.dram_tensor(f"out{u}", [P["tot_rows"], HIDDEN], F16,
                                      kind="ExternalOutput")

    with tile.TileContext(nc) as tc:
        with contextlib.ExitStack() as ctx:
            C = _emit_const(nc, tc, ctx, P, T)
            pools = (
                ctx.enter_context(tc.tile_pool(name="bufs", bufs=1)),
                ctx.enter_context(tc.tile_pool(name="outp", bufs=1)),
                ctx.enter_context(tc.tile_pool(name="psum_mm", bufs=6, space="PSUM")),
            )
            if repeat == 1:
                _emit_body(nc, tc, ctx, P, T, C, pools)
            else:
                stag = bool(K_STAGGER) and unroll == 4
                with tc.For_i(0, repeat // unroll, staggered_reset=stag):
                    for u in range(unroll):
                        if u and stag:
                            tc.stage_boundary()
                        _emit_body(nc, tc, ctx, P, T, C, pools,
                                   sfx="" if u == 0 else f"_u{u}",
                                   out_key=f"out{u}")
    nc.compile()
    return nc


def _weights_maps(head_weight, head_weight_proj, tail_weight_proj_0,
                  tail_weight_0, tail_weight_proj_1, tail_weight_1):
    head_weight = np.asarray(head_weight, np.float32)
    head_weight_proj = np.asarray(head_weight_proj, np.float32)
    scale = np.float32(HIDDEN ** 0.5)
    # head: fold projection + emb scale into the table
    tproj = np.ascontiguousarray(
        (head_weight[:, :HEAD_V].T @ head_weight_proj.T) * scale
    ).astype(np.float16)
    t0 = np.asarray(tail_weight_0, np.float32).T.astype(np.float16)  # [60000,256]
    t1 = np.asarray(tail_weight_1, np.float32).T.astype(np.float16)  # [120000,64]
    t1 = np.concatenate([t1, np.zeros((1, 64), np.float16)], axis=0)
    p1 = (np.asarray(tail_weight_proj_0, np.float32).T * scale).astype(np.float16)
    p1 = np.ascontiguousarray(p1.reshape(2, 128, HIDDEN).transpose(1, 0, 2))
    p2 = np.ascontiguousarray(
        np.asarray(tail_weight_proj_1, np.float32).T * scale).astype(np.float16)
    t0_half = [np.ascontiguousarray(t0[30000 * h:30000 * (h + 1)])
               for h in range(2)]
    t1_var = [np.ascontiguousarray(
        t1.reshape(-1)[64 * (60000 * (v // 2) + (v % 2)):][:30000 * 128]
        .reshape(30000, 128)) for v in range(4)]
    return dict(tproj=tproj, p1=p1, p2=p2, t0_half=t0_half, t1_var=t1_var)


def _assemble(P, results, x_shape):
    n_tok = int(np.prod(x_shape[:2])) if len(x_shape) > 1 else x_shape[0]
    y = np.zeros((n_tok, HIDDEN), np.float32)
    for c in range(N_CORES):
        o = np.asarray(results[c]["out0"], np.float32)
        for s in range(3):
            toks = P["seg_tok"][s][c]
            if len(toks):
                b = P["row_base"][s]
                y[toks] = o[b:b + len(toks)]
    return y.reshape(*x_shape, HIDDEN)


_CACHE = {}


def _get_program(P, repeat=1):
    key = (tuple(P["npad"]), tuple(P["nmax"]), repeat,
           K_WARM_MM, K_UNROLL, K_SINGLE_PACKET, K_STAGGER,
           K_SCRATCH, K_COPY_ALT, K_PROBE, K_FILL, K_QROT)
    if key not in _CACHE:
        _CACHE[key] = _build(P, repeat=repeat)
    return _CACHE[key]


def kernel(x, head_weight, head_weight_proj, tail_weight_proj_0,
           tail_weight_0, tail_weight_proj_1, tail_weight_1):
    x = np.asarray(x)
    P = _plan(x)
    nc = _get_program(P)
    w = _weights_maps(head_weight, head_weight_proj, tail_weight_proj_0,
                      tail_weight_0, tail_weight_proj_1, tail_weight_1)
    in_maps = [dict(tproj=w["tproj"], p1=w["p1"], p2=w["p2"],
                    t0b=w["t0_half"][c % 2], t1v=w["t1_var"][c // 2],
                    idx=P["idx"][c]) for c in range(N_CORES)]
    res = run_bass_kernel_spmd(nc, in_maps, core_ids=list(range(N_CORES)))
    return _assemble(P, res.results, x.shape)
